# revision 21
# baseline (speedup 1.0000x reference)
"""Distributed Trainium2 Bass kernel for one dense transformer block.

Reference computation (B=1, T=2048, D=1024, H=16, HS=64, FF=4096, fp32):
    xn  = rmsnorm(x, g1)
    q,k,v per head; causal softmax attention; sa = attn @ Wproj + bproj
    x   = x + sa
    xn2 = rmsnorm(x, g2)
    x   = x + silu(xn2 @ W1) @ W2

Sharding across 8 NeuronCores:
  - Attention is head-sharded (2 heads/core over the full sequence).
  - Per-head attention keeps keys on the partition axis: sT = kT-block.T @ qT,
    p = exp(sT*scale) (no max subtraction needed -- scores are O(1)), and
    attnT = [v | 1].T @ p accumulated over key tiles, which yields both the
    unnormalized attention output and the softmax denominator in one PSUM
    accumulation chain.  Normalization multiplies by a GpSimd
    partition-broadcast of the reciprocal denominators.
  - QKV runs on raw (unnormalized) x; rstd is folded in at PSUM evacuation,
    so the rmsnorm statistics are off the critical path.
  - One AllToAll redistributes attnT from head-sharded to sequence-sharded
    layout ([1024 features, 256 rows] per core).
  - proj / residual / rmsnorm2 / FFN run sequence-sharded (256 rows/core)
    with replicated Wproj/W1/W2 streamed from HBM.
  - g1/g2 are folded into Wq/Wk/Wv/W1 on the host; bproj is added via a
    rank-1 matmul into the proj PSUM accumulation.
  - PE-facing tensors are fp16 (10-bit mantissa, ~4e-4 matmul rel err,
    full-rate matmul + fast weight load + half DMA); residual adds and
    softmax/norm statistics stay fp32.
  - All weight/activation tensors are pre-shuffled on the host into
    [128-partition, ...] layouts so every DMA is large and contiguous.

Each core returns its 256-row chunk; the host concatenates.
"""

import numpy as np
from contextlib import ExitStack

import concourse.bass as bass
import concourse.tile as tile
from concourse import bacc, mybir
from concourse import bass_utils

T, D, H, HS, FF = 2048, 1024, 16, 64, 4096
NCORES = 8
HPC = H // NCORES      # heads per core = 2
CH = T // NCORES       # rows per core = 256
QB = 512               # query block
NB = T // QB           # 4 query blocks
ND = D // 128          # 8 contraction tiles
NT = T // 128          # 16 key tiles
NG = FF // 512         # 8 FFN column groups
EPS = 1e-6
SCALE = HS ** -0.5

F32 = mybir.dt.float32
F16 = mybir.dt.float16
AF = mybir.ActivationFunctionType
ALU = mybir.AluOpType

_CACHE = {}


def build_nc():
    if "nc" in _CACHE:
        return _CACHE["nc"]

    nc = bacc.Bacc("TRN2", target_bir_lowering=False, debug=False, num_devices=NCORES)

    # All layouts are host-pre-shuffled to [128, ...] partition-major.
    xT_d = nc.dram_tensor("xT", [128, ND, T], F16, kind="ExternalInput")
    xch_d = nc.dram_tensor("xch", [CH, D], F32, kind="ExternalInput")
    wq_d = nc.dram_tensor("wq", [128, ND, HPC * HS], F16, kind="ExternalInput")
    wk_d = nc.dram_tensor("wk", [128, ND, HPC * HS], F16, kind="ExternalInput")
    wv_d = nc.dram_tensor("wv", [128, ND, HPC * HS], F16, kind="ExternalInput")
    wp_d = nc.dram_tensor("wp", [128, ND, D], F16, kind="ExternalInput")
    bp_d = nc.dram_tensor("bp", [1, D], F16, kind="ExternalInput")
    w1_d = nc.dram_tensor("w1", [128, NG, ND, 512], F16, kind="ExternalInput")
    w2_d = nc.dram_tensor("w2", [128, NG, 4, D], F16, kind="ExternalInput")
    ident_d = nc.dram_tensor("ident", [128, 128], F16, kind="ExternalInput")
    ones_c_d = nc.dram_tensor("ones_c", [128, 1], F16, kind="ExternalInput")
    ones_r_d = nc.dram_tensor("ones_r", [1, 128], F16, kind="ExternalInput")
    onescol_d = nc.dram_tensor("onescol", [128, NT], F16, kind="ExternalInput")
    epsb_d = nc.dram_tensor("epsb", [128, 1], F32, kind="ExternalInput")
    out_d = nc.dram_tensor("out", [CH, D], F32, kind="ExternalOutput")

    with tile.TileContext(nc) as tc, ExitStack() as top:
        pers = top.enter_context(tc.tile_pool(name="pers", bufs=1))
        p2 = top.enter_context(tc.tile_pool(name="p2", bufs=2))
        dram = top.enter_context(tc.tile_pool(name="dram", bufs=1, space="DRAM"))

        qT = pers.tile([128, T], F16, name="qT")
        kT = pers.tile([128, T], F16, name="kT")
        va = [pers.tile([128, NT, HS + 1], F16, name=f"va{h}") for h in range(HPC)]
        bnc_i = [dram.tile([NCORES * 128, 128], F16, name=f"bi{m}") for m in range(2)]
        bnc_o = [dram.tile([NCORES * 128, 128], F16, name=f"bo{m}") for m in range(2)]

        # ============ Phase A+B: rmsnorm1 stats + QKV^T =========================
        with ExitStack() as ph:
            pab = ph.enter_context(tc.tile_pool(name="pab", bufs=1))
            pstr = ph.enter_context(tc.tile_pool(name="pstr", bufs=3))
            psq = ph.enter_context(tc.tile_pool(name="psq", bufs=2, space="PSUM"))
            pss = ph.enter_context(tc.tile_pool(name="pss", bufs=1, space="PSUM"))
            ptp = ph.enter_context(tc.tile_pool(name="ptp", bufs=1, space="PSUM"))

            # x first -- everything depends on it; then the small QKV weights.
            xt3 = pab.tile([128, ND, T], F16, name="xt3")
            nc.sync.dma_start(xt3[:], xT_d[:])
            xt = [xt3[:, d, :] for d in range(ND)]
            wq3 = pab.tile([128, ND, HPC * HS], F16, name="wq3")
            wk3 = pab.tile([128, ND, HPC * HS], F16, name="wk3")
            wv3 = pab.tile([128, ND, HPC * HS], F16, name="wv3")
            nc.sync.dma_start(wq3[:], wq_d[:])
            nc.sync.dma_start(wk3[:], wk_d[:])
            nc.sync.dma_start(wv3[:], wv_d[:])
            wq = [wq3[:, d, :] for d in range(ND)]
            wk = [wk3[:, d, :] for d in range(ND)]
            wv = [wv3[:, d, :] for d in range(ND)]

            # small constants (scalar queue; tiny)
            ident = pers.tile([128, 128], F16, name="ident")
            nc.scalar.dma_start(ident[:], ident_d[:])
            ones_c = pers.tile([128, 1], F16, name="ones_c")
            nc.scalar.dma_start(ones_c[:], ones_c_d[:])
            ones_r = pers.tile([1, 128], F16, name="ones_r")
            nc.scalar.dma_start(ones_r[:], ones_r_d[:])
            bp = pers.tile([1, D], F16, name="bp")
            nc.scalar.dma_start(bp[:], bp_d[:])
            epsb = pers.tile([128, 1], F32, name="epsb")
            nc.scalar.dma_start(epsb[:], epsb_d[:])
            for h in range(HPC):
                nc.scalar.dma_start(va[h][:, :, HS], onescol_d[:])

            # proj weights early on the bulk queue (after qkv weights)
            wp3 = pers.tile([128, ND, D], F16, name="wp3")
            nc.sync.dma_start(wp3[:], wp_d[:])
            wp = [wp3[:, f, :] for f in range(ND)]

            # rmsnorm stats: all squares + partition-sums first (DVE for the
            # first half, GpSimd for the second), then the per-block tails, so
            # no engine FIFO blocks another phase's work.
            sss = []
            for tb in range(NB):
                cs = slice(QB * tb, QB * (tb + 1))
                ps_ss = pss.tile([1, QB], F32, name="ps_ss", tag="ps_ss")
                for d in range(ND):
                    sq = pstr.tile([128, QB], F16, name="sq")
                    nc.vector.tensor_mul(sq[:], xt[d][:, cs], xt[d][:, cs])
                    nc.tensor.matmul(
                        ps_ss[:], ones_c[:], sq[:], start=(d == 0), stop=(d == ND - 1)
                    )
                sqr = pstr.tile([1, QB], F32, name="sqr")
                nc.scalar.activation(
                    sqr[:], ps_ss[:], AF.Sqrt, scale=1.0 / D, bias=epsb[0:1, :]
                )
                sss.append(sqr)
            bcs = []
            for tb in range(NB):
                rstd = pstr.tile([1, QB], F32, name="rstd")
                nc.vector.reciprocal_approx_fast(rstd[:], sss[tb][:])
                bc = pab.tile([128, QB], F32, name=f"bc{tb}")
                nc.gpsimd.partition_broadcast(bc[:], rstd[:])
                bcs.append(bc)

            # raw QKV^T; rstd folded in at evacuation
            for tb in range(NB):
                cs = slice(QB * tb, QB * (tb + 1))
                bc = bcs[tb]
                ps_q = psq.tile([128, QB], F32, name="ps_q")
                ps_k = psq.tile([128, QB], F32, name="ps_k")
                ps_v = psq.tile([128, QB], F32, name="ps_v")
                for d in range(ND):
                    st, sp = (d == 0), (d == ND - 1)
                    nc.tensor.matmul(ps_q[:], wq[d], xt[d][:, cs], start=st, stop=sp)
                    nc.tensor.matmul(ps_k[:], wk[d], xt[d][:, cs], start=st, stop=sp)
                    nc.tensor.matmul(ps_v[:], wv[d], xt[d][:, cs], start=st, stop=sp)
                nc.vector.tensor_mul(qT[:, cs], ps_q[:], bc[:])
                nc.vector.tensor_mul(kT[:, cs], ps_k[:], bc[:])
                vt = pstr.tile([128, QB], F16, name="vt")
                nc.vector.tensor_mul(vt[:], ps_v[:], bc[:])
                for s in range(4):
                    tt = 4 * tb + s
                    ps_t = ptp.tile([128, 128], F16, name="ps_t")
                    nc.tensor.transpose(
                        ps_t[:], vt[:, 128 * s : 128 * (s + 1)], ident[:]
                    )
                    for h in range(HPC):
                        nc.scalar.copy(
                            va[h][:, tt, 0:HS], ps_t[:, HS * h : HS * (h + 1)]
                        )

        # ============ Phase C: causal attention ================================
        with ExitStack() as ph:
            ppt = ph.enter_context(tc.tile_pool(name="ppt", bufs=6))
            psat = ph.enter_context(tc.tile_pool(name="psat", bufs=1, space="PSUM"))
            pscs = ph.enter_context(tc.tile_pool(name="pscs", bufs=3, space="PSUM"))

            for qb in range(NB):
                qs = slice(QB * qb, QB * (qb + 1))
                nkt = 4 * (qb + 1)
                ps_at = [
                    psat.tile([HS + 1, QB], F32, name=f"at{h}", tag=f"at{h}")
                    for h in range(HPC)
                ]
                for kt0 in range(0, nkt, 2):
                    for h in range(HPC):
                        hsl = slice(HS * h, HS * (h + 1))
                        ps_s = pscs.tile([128, 1024], F32, name="ps_s", tag="ps_s")
                        for i in range(2):
                            kt = kt0 + i
                            nc.tensor.matmul(
                                ps_s[:, 512 * i : 512 * (i + 1)],
                                kT[hsl, 128 * kt : 128 * (kt + 1)],
                                qT[hsl, qs],
                                start=True,
                                stop=True,
                            )
                        pt = ppt.tile([128, 1024], F16, name="pt")
                        nc.scalar.activation(pt[:], ps_s[:], AF.Exp, scale=SCALE)
                        for i in range(2):
                            kt = kt0 + i
                            if kt >= 4 * qb:  # diagonal tile: zero where k > q
                                nc.gpsimd.affine_select(
                                    pt[:, 512 * i : 512 * (i + 1)],
                                    pt[:, 512 * i : 512 * (i + 1)],
                                    pattern=[[1, 512]],
                                    compare_op=ALU.is_ge,
                                    fill=0.0,
                                    base=QB * qb - 128 * kt,
                                    channel_multiplier=-1,
                                )
                        for i in range(2):
                            kt = kt0 + i
                            nc.tensor.matmul(
                                ps_at[h][:],
                                va[h][:, kt, :],
                                pt[:, 512 * i : 512 * (i + 1)],
                                start=(kt == 0),
                                stop=(kt == nkt - 1),
                            )
                attnT2 = p2.tile([128, QB], F16, name="attnT2")
                for h in range(HPC):
                    hsl = slice(HS * h, HS * (h + 1))
                    den_sb = p2.tile([1, QB], F32, name="den_sb", tag="den_sb")
                    nc.vector.tensor_copy(den_sb[:], ps_at[h][HS : HS + 1, :])
                    recip_h = p2.tile([1, QB], F32, name="recip", tag="recip")
                    nc.vector.reciprocal_approx_fast(recip_h[:], den_sb[:])
                    bc_sb = p2.tile([HS, QB], F32, name="bc_sb", tag="bc_sb")
                    nc.gpsimd.partition_broadcast(bc_sb[:], recip_h[:])
                    nc.vector.tensor_mul(attnT2[hsl, :], ps_at[h][0:HS, :], bc_sb[:])
                m, sph = qb // 2, qb % 2
                for jj in range(4):
                    j = 4 * sph + jj
                    nc.scalar.dma_start(
                        bnc_i[m][128 * j : 128 * (j + 1), :],
                        attnT2[:, 128 * jj : 128 * (jj + 1)],
                    )
                if sph == 1:
                    nc.gpsimd.collective_compute(
                        "AllToAll",
                        ALU.bypass,
                        replica_groups=[list(range(NCORES))],
                        ins=[bnc_i[m].opt()],
                        outs=[bnc_o[m].opt()],
                    )

        # ============ Phase D: proj + residual + rmsnorm2 (+ transpose) ==========
        x2 = [pers.tile([128, D], F32, name=f"x2_{ts}") for ts in range(2)]
        xn2T = [
            [pers.tile([128, 128], F16, name=f"xn2T{d}_{ts}") for ts in range(2)]
            for d in range(ND)
        ]
        with ExitStack() as ph:
            pd = ph.enter_context(tc.tile_pool(name="pd", bufs=1))
            pds = ph.enter_context(tc.tile_pool(name="pds", bufs=2))
            psd1 = ph.enter_context(tc.tile_pool(name="psd1", bufs=2, space="PSUM"))
            psd2 = ph.enter_context(tc.tile_pool(name="psd2", bufs=1, space="PSUM"))

            aT = [
                [pd.tile([128, 128], F16, name=f"aT{f}_{m}") for m in range(2)]
                for f in range(ND)
            ]
            for m in range(2):
                for f in range(ND):
                    nc.scalar.dma_start(
                        aT[f][m][:], bnc_o[m][128 * f : 128 * (f + 1), :]
                    )
            xch = [pd.tile([128, D], F32, name=f"xch{ts}") for ts in range(2)]
            for ts in range(2):
                nc.scalar.dma_start(xch[ts][:], xch_d[128 * ts : 128 * (ts + 1), :])

            for ts in range(2):
                tsl = slice(128 * ts, 128 * (ts + 1))
                for b in range(2):
                    bsl = slice(512 * b, 512 * (b + 1))
                    ps_sa = psd1.tile([128, 512], F32, name="ps_sa", tag="ps_sa")
                    nc.tensor.matmul(
                        ps_sa[:], ones_r[:], bp[0:1, bsl], start=True, stop=False
                    )
                    for f in range(ND):
                        nc.tensor.matmul(
                            ps_sa[:],
                            aT[f][ts][:],
                            wp[f][:, bsl],
                            start=False,
                            stop=(f == ND - 1),
                        )
                    nc.vector.tensor_add(
                        x2[ts][:, bsl], ps_sa[:], xch[ts][:, bsl]
                    )
                sq2 = pds.tile([128, D], F32, name="sq2")
                ss2 = pds.tile([128, 1], F32, name="ss2")
                nc.scalar.activation(sq2[:], x2[ts][:], AF.Square, accum_out=ss2[:])
                sqr2 = pds.tile([128, 1], F32, name="sqr2")
                nc.scalar.activation(
                    sqr2[:], ss2[:], AF.Sqrt, scale=1.0 / D, bias=epsb[:]
                )
                rstd2 = pds.tile([128, 1], F32, name="rstd2")
                nc.vector.reciprocal(rstd2[:], sqr2[:])
                xn2 = pds.tile([128, D], F16, name="xn2")
                with nc.allow_low_precision("fp16 rounding is intended"):
                    nc.scalar.activation(xn2[:], x2[ts][:], AF.Copy, scale=rstd2[:])
                for d in range(ND):
                    ps_t2 = psd2.tile([128, 128], F16, name="ps_t2")
                    nc.tensor.transpose(
                        ps_t2[:], xn2[:, 128 * d : 128 * (d + 1)], ident[:]
                    )
                    nc.vector.tensor_copy(xn2T[d][ts][:], ps_t2[:])

        # ============ Phase E: FFN =============================================
        # W1 half is split by row-half (ts) so the ts=0 half can run as soon as
        # the first AllToAll lands, overlapping the second one.  w1 is fully
        # resident; hT is kept for the separate W2 pass.
        with ExitStack() as ph:
            pw1 = ph.enter_context(tc.tile_pool(name="pw1", bufs=1))
            pw2 = ph.enter_context(tc.tile_pool(name="pw2", bufs=2))
            pht = ph.enter_context(tc.tile_pool(name="pht", bufs=1))
            pse1 = ph.enter_context(tc.tile_pool(name="pse1", bufs=1, space="PSUM"))
            pse3 = ph.enter_context(tc.tile_pool(name="pse3", bufs=2, space="PSUM"))

            w1g = []
            for g in range(NG):
                t_ = pw1.tile([128, ND, 512], F16, name=f"w1g{g}")
                nc.sync.dma_start(t_[:], w1_d[:, g])
                w1g.append(t_)
            hTs = [pht.tile([128, FF // 128, 128], F16, name=f"hT{ts}") for ts in range(2)]
            ps_out = [pse1.tile([128, D], F32, name=f"ps_out{ts}") for ts in range(2)]

            for ts in range(2):
                tsl = slice(128 * ts, 128 * (ts + 1))
                for g in range(NG):
                    for j in range(4):
                        ff = 4 * g + j
                        ps_h = pse3.tile([128, 128], F32, name="ps_h", tag="ps_h")
                        for d in range(ND):
                            nc.tensor.matmul(
                                ps_h[:],
                                w1g[g][:, d, 128 * j : 128 * (j + 1)],
                                xn2T[d][ts][:],
                                start=(d == 0),
                                stop=(d == ND - 1),
                            )
                        nc.scalar.activation(hTs[ts][:, ff, :], ps_h[:], AF.Silu)

            for g in range(NG):
                w2g = pw2.tile([128, 4, D], F16, name="w2g", tag="w2g")
                nc.sync.dma_start(w2g[:], w2_d[:, g])
                for j in range(4):
                    ff = 4 * g + j
                    for ts in range(2):
                        for b in range(2):
                            nc.tensor.matmul(
                                ps_out[ts][:, 512 * b : 512 * (b + 1)],
                                hTs[ts][:, ff, :],
                                w2g[:, j, 512 * b : 512 * (b + 1)],
                                start=(ff == 0),
                                stop=(ff == FF // 128 - 1),
                            )
            for ts in range(2):
                out_t = pw2.tile([128, D], F32, name=f"out{ts}", tag=f"out{ts}")
                nc.vector.tensor_add(out_t[:], ps_out[ts][:], x2[ts][:])
                nc.scalar.dma_start(out_d[128 * ts : 128 * (ts + 1), :], out_t[:])

    nc.compile()
    _CACHE["nc"] = nc
    return nc


def _shuf(a, p=128):
    """[N*p, C] -> [p, N, C] partition-major host shuffle."""
    n = a.shape[0] // p
    return np.ascontiguousarray(a.reshape(n, p, -1).transpose(1, 0, 2))


def make_in_maps(inputs):
    x = np.asarray(inputs["x"], np.float32).reshape(T, D)
    Wq = np.asarray(inputs["Wq"], np.float32)
    Wk = np.asarray(inputs["Wk"], np.float32)
    Wv = np.asarray(inputs["Wv"], np.float32)
    Wproj = np.asarray(inputs["Wproj"], np.float32)
    bproj = np.asarray(inputs["bproj"], np.float32).reshape(1, D)
    W1 = np.asarray(inputs["W1"], np.float32)
    W2 = np.asarray(inputs["W2"], np.float32)
    g1 = np.asarray(inputs["g1"], np.float32)
    g2 = np.asarray(inputs["g2"], np.float32)

    xT = _shuf(np.ascontiguousarray(x.T).astype(np.float16))          # [128,8,T]
    Wq_f = (Wq * g1[None, :, None]).astype(np.float16)
    Wk_f = (Wk * g1[None, :, None]).astype(np.float16)
    Wv_f = (Wv * g1[None, :, None]).astype(np.float16)
    Wp_s = _shuf(Wproj.astype(np.float16))                            # [128,8,D]
    W1_f = (W1 * g2[:, None]).astype(np.float16)
    # w1: [p, g, d, 512] so each g-block is one contiguous DMA
    w1_s = np.ascontiguousarray(
        W1_f.reshape(ND, 128, NG, 512).transpose(1, 2, 0, 3)
    )
    # w2: [p, g, j, D]
    w2_s = np.ascontiguousarray(
        W2.astype(np.float16).reshape(NG, 4, 128, D).transpose(2, 0, 1, 3)
    )

    common = {
        "xT": xT,
        "wp": Wp_s,
        "bp": bproj.astype(np.float16),
        "w1": w1_s,
        "w2": w2_s,
        "ident": np.eye(128, dtype=np.float16),
        "ones_c": np.ones((128, 1), np.float16),
        "ones_r": np.ones((1, 128), np.float16),
        "onescol": np.ones((128, NT), np.float16),
        "epsb": np.full((128, 1), EPS, np.float32),
    }
    in_maps = []
    for c in range(NCORES):
        heads = [HPC * c + h for h in range(HPC)]
        in_maps.append(
            {
                **common,
                "xch": np.ascontiguousarray(x[_chunk_rows(c)]),
                "wq": _shuf(np.concatenate([Wq_f[h] for h in heads], 1)),
                "wk": _shuf(np.concatenate([Wk_f[h] for h in heads], 1)),
                "wv": _shuf(np.concatenate([Wv_f[h] for h in heads], 1)),
            }
        )
    return in_maps


def _chunk_rows(j):
    """Core j owns rows [1024m + 128j : +128) for m = 0, 1."""
    return np.concatenate(
        [np.arange(1024 * m + 128 * j, 1024 * m + 128 * (j + 1)) for m in range(2)]
    )


def run(inputs, **kwargs):
    nc = build_nc()
    in_maps = make_in_maps(inputs)
    res = bass_utils.run_bass_kernel_spmd(
        nc, in_maps, core_ids=list(range(NCORES)), **kwargs
    )
    out = np.empty((T, D), np.float32)
    for c in range(NCORES):
        out[_chunk_rows(c)] = res.results[c]["out"]
    return out.reshape(1, T, D), res


def kernel(**inputs):
    out, _ = run(inputs)
    return out


# revision 22
# speedup vs baseline: 1.0725x; 1.0725x over previous
"""Distributed Trainium2 Bass kernel for one dense transformer block.

Reference computation (B=1, T=2048, D=1024, H=16, HS=64, FF=4096, fp32):
    xn  = rmsnorm(x, g1)
    q,k,v per head; causal softmax attention; sa = attn @ Wproj + bproj
    x   = x + sa
    xn2 = rmsnorm(x, g2)
    x   = x + silu(xn2 @ W1) @ W2

Sharding across 8 NeuronCores:
  - Attention is head-sharded (2 heads/core over the full sequence).
  - Per-head attention keeps keys on the partition axis: sT = kT-block.T @ qT,
    p = exp(sT*scale) (no max subtraction needed -- scores are O(1)), and
    attnT = [v | 1].T @ p accumulated over key tiles, which yields both the
    unnormalized attention output and the softmax denominator in one PSUM
    accumulation chain.  Normalization multiplies by a GpSimd
    partition-broadcast of the reciprocal denominators.
  - QKV runs on raw (unnormalized) x; rstd is folded in at PSUM evacuation,
    so the rmsnorm statistics are off the critical path.
  - One AllToAll redistributes attnT from head-sharded to sequence-sharded
    layout ([1024 features, 256 rows] per core).
  - proj / residual / rmsnorm2 / FFN run sequence-sharded (256 rows/core)
    with replicated Wproj/W1/W2 streamed from HBM.
  - g1/g2 are folded into Wq/Wk/Wv/W1 on the host; bproj is added via a
    rank-1 matmul into the proj PSUM accumulation.
  - PE-facing tensors are fp16 (10-bit mantissa, ~4e-4 matmul rel err,
    full-rate matmul + fast weight load + half DMA); residual adds and
    softmax/norm statistics stay fp32.
  - All weight/activation tensors are pre-shuffled on the host into
    [128-partition, ...] layouts so every DMA is large and contiguous.

Each core returns its 256-row chunk; the host concatenates.
"""

import numpy as np
from contextlib import ExitStack

import concourse.bass as bass
import concourse.tile as tile
from concourse import bacc, mybir
from concourse import bass_utils

T, D, H, HS, FF = 2048, 1024, 16, 64, 4096
NCORES = 8
HPC = H // NCORES      # heads per core = 2
CH = T // NCORES       # rows per core = 256
QB = 512               # query block
NB = T // QB           # 4 query blocks
ND = D // 128          # 8 contraction tiles
NT = T // 128          # 16 key tiles
NG = FF // 512         # 8 FFN column groups
EPS = 1e-6
SCALE = HS ** -0.5

F32 = mybir.dt.float32
F16 = mybir.dt.float16
AF = mybir.ActivationFunctionType
ALU = mybir.AluOpType

_CACHE = {}


def build_nc():
    if "nc" in _CACHE:
        return _CACHE["nc"]

    nc = bacc.Bacc("TRN2", target_bir_lowering=False, debug=False, num_devices=NCORES)

    # All layouts are host-pre-shuffled to [128, ...] partition-major.
    xT_d = nc.dram_tensor("xT", [128, ND, T], F16, kind="ExternalInput")
    xch_d = nc.dram_tensor("xch", [CH, D], F32, kind="ExternalInput")
    wq_d = nc.dram_tensor("wq", [128, ND, HPC * HS], F16, kind="ExternalInput")
    wk_d = nc.dram_tensor("wk", [128, ND, HPC * HS], F16, kind="ExternalInput")
    wv_d = nc.dram_tensor("wv", [128, ND, HPC * HS], F16, kind="ExternalInput")
    wp_d = nc.dram_tensor("wp", [128, ND, D], F16, kind="ExternalInput")
    bp_d = nc.dram_tensor("bp", [1, D], F16, kind="ExternalInput")
    w1_d = nc.dram_tensor("w1", [128, NG, ND, 512], F16, kind="ExternalInput")
    w2_d = nc.dram_tensor("w2", [128, NG, 4, D], F16, kind="ExternalInput")
    ident_d = nc.dram_tensor("ident", [128, 128], F16, kind="ExternalInput")
    ones_c_d = nc.dram_tensor("ones_c", [128, 1], F16, kind="ExternalInput")
    ones_r_d = nc.dram_tensor("ones_r", [1, 128], F16, kind="ExternalInput")
    onescol_d = nc.dram_tensor("onescol", [128, NT], F16, kind="ExternalInput")
    epsb_d = nc.dram_tensor("epsb", [128, 1], F32, kind="ExternalInput")
    out_d = nc.dram_tensor("out", [CH, D], F32, kind="ExternalOutput")

    with tile.TileContext(nc) as tc, ExitStack() as top:
        pers = top.enter_context(tc.tile_pool(name="pers", bufs=1))
        p2 = top.enter_context(tc.tile_pool(name="p2", bufs=2))
        dram = top.enter_context(tc.tile_pool(name="dram", bufs=1, space="DRAM"))

        qT = pers.tile([128, T], F16, name="qT")
        kT = pers.tile([128, T], F16, name="kT")
        va = [pers.tile([128, NT, HS + 1], F16, name=f"va{h}") for h in range(HPC)]
        bnc_i = [dram.tile([NCORES * 128, 128], F16, name=f"bi{m}") for m in range(2)]
        bnc_o = [dram.tile([NCORES * 128, 128], F16, name=f"bo{m}") for m in range(2)]

        # ============ Phase A+B: rmsnorm1 stats + QKV^T =========================
        with ExitStack() as ph:
            pab = ph.enter_context(tc.tile_pool(name="pab", bufs=1))
            pstr = ph.enter_context(tc.tile_pool(name="pstr", bufs=3))
            psq = ph.enter_context(tc.tile_pool(name="psq", bufs=2, space="PSUM"))
            pss = ph.enter_context(tc.tile_pool(name="pss", bufs=1, space="PSUM"))
            ptp = ph.enter_context(tc.tile_pool(name="ptp", bufs=1, space="PSUM"))

            # x first -- everything depends on it; then the small QKV weights.
            xt3 = pab.tile([128, ND, T], F16, name="xt3")
            nc.sync.dma_start(xt3[:], xT_d[:])
            xt = [xt3[:, d, :] for d in range(ND)]
            wq3 = pab.tile([128, ND, HPC * HS], F16, name="wq3")
            wk3 = pab.tile([128, ND, HPC * HS], F16, name="wk3")
            wv3 = pab.tile([128, ND, HPC * HS], F16, name="wv3")
            nc.sync.dma_start(wq3[:], wq_d[:])
            nc.sync.dma_start(wk3[:], wk_d[:])
            nc.sync.dma_start(wv3[:], wv_d[:])
            wq = [wq3[:, d, :] for d in range(ND)]
            wk = [wk3[:, d, :] for d in range(ND)]
            wv = [wv3[:, d, :] for d in range(ND)]

            # small constants (scalar queue; tiny)
            ident = pers.tile([128, 128], F16, name="ident")
            nc.scalar.dma_start(ident[:], ident_d[:])
            ones_c = pers.tile([128, 1], F16, name="ones_c")
            nc.scalar.dma_start(ones_c[:], ones_c_d[:])
            ones_r = pers.tile([1, 128], F16, name="ones_r")
            nc.scalar.dma_start(ones_r[:], ones_r_d[:])
            bp = pers.tile([1, D], F16, name="bp")
            nc.scalar.dma_start(bp[:], bp_d[:])
            epsb = pers.tile([128, 1], F32, name="epsb")
            nc.scalar.dma_start(epsb[:], epsb_d[:])
            for h in range(HPC):
                nc.scalar.dma_start(va[h][:, :, HS], onescol_d[:])

            # proj weights early on the bulk queue (after qkv weights)
            wp3 = pers.tile([128, ND, D], F16, name="wp3")
            nc.sync.dma_start(wp3[:], wp_d[:])
            wp = [wp3[:, f, :] for f in range(ND)]

            # rmsnorm stats: all squares + partition-sums first (DVE for the
            # first half, GpSimd for the second), then the per-block tails, so
            # no engine FIFO blocks another phase's work.
            sss = []
            for tb in range(NB):
                cs = slice(QB * tb, QB * (tb + 1))
                ps_ss = pss.tile([1, QB], F32, name="ps_ss", tag="ps_ss")
                for d in range(ND):
                    sq = pstr.tile([128, QB], F16, name="sq")
                    nc.vector.tensor_mul(sq[:], xt[d][:, cs], xt[d][:, cs])
                    nc.tensor.matmul(
                        ps_ss[:], ones_c[:], sq[:], start=(d == 0), stop=(d == ND - 1)
                    )
                sqr = pstr.tile([1, QB], F32, name="sqr")
                nc.scalar.activation(
                    sqr[:], ps_ss[:], AF.Sqrt, scale=1.0 / D, bias=epsb[0:1, :]
                )
                sss.append(sqr)
            bcs = []
            for tb in range(NB):
                rstd = pstr.tile([1, QB], F32, name="rstd")
                nc.vector.reciprocal_approx_fast(rstd[:], sss[tb][:])
                bc = pab.tile([128, QB], F32, name=f"bc{tb}")
                nc.gpsimd.partition_broadcast(bc[:], rstd[:])
                bcs.append(bc)

            # raw QKV^T; rstd folded in at evacuation
            for tb in range(NB):
                cs = slice(QB * tb, QB * (tb + 1))
                bc = bcs[tb]
                ps_q = psq.tile([128, QB], F32, name="ps_q")
                ps_k = psq.tile([128, QB], F32, name="ps_k")
                ps_v = psq.tile([128, QB], F32, name="ps_v")
                for d in range(ND):
                    st, sp = (d == 0), (d == ND - 1)
                    nc.tensor.matmul(ps_q[:], wq[d], xt[d][:, cs], start=st, stop=sp)
                    nc.tensor.matmul(ps_k[:], wk[d], xt[d][:, cs], start=st, stop=sp)
                    nc.tensor.matmul(ps_v[:], wv[d], xt[d][:, cs], start=st, stop=sp)
                nc.vector.tensor_mul(qT[:, cs], ps_q[:], bc[:])
                nc.vector.tensor_mul(kT[:, cs], ps_k[:], bc[:])
                vt = pstr.tile([128, QB], F16, name="vt")
                nc.vector.tensor_mul(vt[:], ps_v[:], bc[:])
                for s in range(4):
                    tt = 4 * tb + s
                    ps_t = ptp.tile([128, 128], F16, name="ps_t")
                    nc.tensor.transpose(
                        ps_t[:], vt[:, 128 * s : 128 * (s + 1)], ident[:]
                    )
                    for h in range(HPC):
                        nc.scalar.copy(
                            va[h][:, tt, 0:HS], ps_t[:, HS * h : HS * (h + 1)]
                        )

        # ============ Phase C: causal attention ================================
        with ExitStack() as ph:
            ppt = ph.enter_context(tc.tile_pool(name="ppt", bufs=6))
            psat = ph.enter_context(tc.tile_pool(name="psat", bufs=1, space="PSUM"))
            pscs = ph.enter_context(tc.tile_pool(name="pscs", bufs=3, space="PSUM"))

            for qb in range(NB):
                qs = slice(QB * qb, QB * (qb + 1))
                nkt = 4 * (qb + 1)
                ps_at = [
                    psat.tile([HS + 1, QB], F32, name=f"at{h}", tag=f"at{h}")
                    for h in range(HPC)
                ]
                for kt0 in range(0, nkt, 2):
                    for h in range(HPC):
                        hsl = slice(HS * h, HS * (h + 1))
                        ps_s = pscs.tile([128, 1024], F32, name="ps_s", tag="ps_s")
                        for i in range(2):
                            kt = kt0 + i
                            nc.tensor.matmul(
                                ps_s[:, 512 * i : 512 * (i + 1)],
                                kT[hsl, 128 * kt : 128 * (kt + 1)],
                                qT[hsl, qs],
                                start=True,
                                stop=True,
                            )
                        pt = ppt.tile([128, 1024], F16, name="pt")
                        nc.scalar.activation(pt[:], ps_s[:], AF.Exp, scale=SCALE)
                        for i in range(2):
                            kt = kt0 + i
                            if kt >= 4 * qb:  # diagonal tile: zero where k > q
                                nc.gpsimd.affine_select(
                                    pt[:, 512 * i : 512 * (i + 1)],
                                    pt[:, 512 * i : 512 * (i + 1)],
                                    pattern=[[1, 512]],
                                    compare_op=ALU.is_ge,
                                    fill=0.0,
                                    base=QB * qb - 128 * kt,
                                    channel_multiplier=-1,
                                )
                        for i in range(2):
                            kt = kt0 + i
                            nc.tensor.matmul(
                                ps_at[h][:],
                                va[h][:, kt, :],
                                pt[:, 512 * i : 512 * (i + 1)],
                                start=(kt == 0),
                                stop=(kt == nkt - 1),
                            )
                attnT2 = p2.tile([128, QB], F16, name="attnT2")
                for h in range(HPC):
                    hsl = slice(HS * h, HS * (h + 1))
                    den_sb = p2.tile([1, QB], F32, name="den_sb", tag="den_sb")
                    nc.vector.tensor_copy(den_sb[:], ps_at[h][HS : HS + 1, :])
                    recip_h = p2.tile([1, QB], F32, name="recip", tag="recip")
                    nc.vector.reciprocal_approx_fast(recip_h[:], den_sb[:])
                    bc_sb = p2.tile([HS, QB], F32, name="bc_sb", tag="bc_sb")
                    nc.gpsimd.partition_broadcast(bc_sb[:], recip_h[:])
                    nc.vector.tensor_mul(attnT2[hsl, :], ps_at[h][0:HS, :], bc_sb[:])
                m, sph = qb // 2, qb % 2
                for jj in range(4):
                    j = 4 * sph + jj
                    nc.scalar.dma_start(
                        bnc_i[m][128 * j : 128 * (j + 1), :],
                        attnT2[:, 128 * jj : 128 * (jj + 1)],
                    )
                if sph == 1:
                    nc.gpsimd.collective_compute(
                        "AllToAll",
                        ALU.bypass,
                        replica_groups=[list(range(NCORES))],
                        ins=[bnc_i[m].opt()],
                        outs=[bnc_o[m].opt()],
                    )

        # ============ Phase D+E: per-row-half pipelines ========================
        # Everything for row-half ts=0 (proj, rmsnorm2, FFN W1 half) is emitted
        # before any ts=1 work, so the PE stream never blocks on the second
        # AllToAll while ts=0 work is available.
        x2 = [pers.tile([128, D], F32, name=f"x2_{ts}") for ts in range(2)]
        with ExitStack() as ph:
            pd = ph.enter_context(tc.tile_pool(name="pd", bufs=1))
            pds = ph.enter_context(tc.tile_pool(name="pds", bufs=2))
            pw1 = ph.enter_context(tc.tile_pool(name="pw1", bufs=1))
            pw2 = ph.enter_context(tc.tile_pool(name="pw2", bufs=2))
            psd1 = ph.enter_context(tc.tile_pool(name="psd1", bufs=1, space="PSUM"))
            psd2 = ph.enter_context(tc.tile_pool(name="psd2", bufs=1, space="PSUM"))
            pse1 = ph.enter_context(tc.tile_pool(name="pse1", bufs=1, space="PSUM"))
            pse3 = ph.enter_context(tc.tile_pool(name="pse3", bufs=2, space="PSUM"))

            w1g = []
            for g in range(NG):
                t_ = pw1.tile([128, ND, 512], F16, name=f"w1g{g}")
                nc.sync.dma_start(t_[:], w1_d[:, g])
                w1g.append(t_)
            hTs = [
                pd.tile([128, FF // 128, 128], F16, name=f"hT{ts}") for ts in range(2)
            ]
            ps_out = [pse1.tile([128, D], F32, name=f"ps_out{ts}") for ts in range(2)]

            aT = [
                [pd.tile([128, 128], F16, name=f"aT{f}_{m}") for m in range(2)]
                for f in range(ND)
            ]
            xch = [pd.tile([128, D], F32, name=f"xch{ts}") for ts in range(2)]

            for ts in range(2):
                tsl = slice(128 * ts, 128 * (ts + 1))
                for f in range(ND):
                    nc.scalar.dma_start(
                        aT[f][ts][:], bnc_o[ts][128 * f : 128 * (f + 1), :]
                    )
                nc.scalar.dma_start(xch[ts][:], xch_d[128 * ts : 128 * (ts + 1), :])

                # proj + bias + residual, one 512-column bank at a time
                for b in range(2):
                    bsl = slice(512 * b, 512 * (b + 1))
                    ps_sa = psd1.tile([128, 512], F32, name="ps_sa", tag="ps_sa")
                    nc.tensor.matmul(
                        ps_sa[:], ones_r[:], bp[0:1, bsl], start=True, stop=False
                    )
                    for f in range(ND):
                        nc.tensor.matmul(
                            ps_sa[:],
                            aT[f][ts][:],
                            wp[f][:, bsl],
                            start=False,
                            stop=(f == ND - 1),
                        )
                    nc.vector.tensor_add(x2[ts][:, bsl], ps_sa[:], xch[ts][:, bsl])

                # rmsnorm2
                sq2 = pds.tile([128, D], F32, name="sq2")
                ss2 = pds.tile([128, 1], F32, name="ss2")
                nc.scalar.activation(sq2[:], x2[ts][:], AF.Square, accum_out=ss2[:])
                sqr2 = pds.tile([128, 1], F32, name="sqr2")
                nc.scalar.activation(
                    sqr2[:], ss2[:], AF.Sqrt, scale=1.0 / D, bias=epsb[:]
                )
                rstd2 = pds.tile([128, 1], F32, name="rstd2")
                nc.vector.reciprocal(rstd2[:], sqr2[:])
                xn2 = pds.tile([128, D], F16, name="xn2")
                with nc.allow_low_precision("fp16 rounding is intended"):
                    nc.scalar.activation(xn2[:], x2[ts][:], AF.Copy, scale=rstd2[:])
                xn2T = []
                for d in range(ND):
                    ps_t2 = psd2.tile([128, 128], F16, name="ps_t2", tag="ps_t2")
                    nc.tensor.transpose(
                        ps_t2[:], xn2[:, 128 * d : 128 * (d + 1)], ident[:]
                    )
                    xt2 = pd.tile([128, 128], F16, name=f"xn2T{d}_{ts}")
                    nc.vector.tensor_copy(xt2[:], ps_t2[:])
                    xn2T.append(xt2)

                # FFN W1 half for this row half
                for g in range(NG):
                    for j in range(4):
                        ff = 4 * g + j
                        ps_h = pse3.tile([128, 128], F32, name="ps_h", tag="ps_h")
                        for d in range(ND):
                            nc.tensor.matmul(
                                ps_h[:],
                                w1g[g][:, d, 128 * j : 128 * (j + 1)],
                                xn2T[d][:],
                                start=(d == 0),
                                stop=(d == ND - 1),
                            )
                        nc.scalar.activation(hTs[ts][:, ff, :], ps_h[:], AF.Silu)

            # FFN W2 pass
            for g in range(NG):
                w2g = pw2.tile([128, 4, D], F16, name="w2g", tag="w2g")
                nc.sync.dma_start(w2g[:], w2_d[:, g])
                for j in range(4):
                    ff = 4 * g + j
                    for ts in range(2):
                        for b in range(2):
                            nc.tensor.matmul(
                                ps_out[ts][:, 512 * b : 512 * (b + 1)],
                                hTs[ts][:, ff, :],
                                w2g[:, j, 512 * b : 512 * (b + 1)],
                                start=(ff == 0),
                                stop=(ff == FF // 128 - 1),
                            )
            for ts in range(2):
                out_t = pw2.tile([128, D], F32, name=f"out{ts}", tag=f"out{ts}")
                nc.vector.tensor_add(out_t[:], ps_out[ts][:], x2[ts][:])
                nc.scalar.dma_start(out_d[128 * ts : 128 * (ts + 1), :], out_t[:])

    nc.compile()
    _CACHE["nc"] = nc
    return nc


def _shuf(a, p=128):
    """[N*p, C] -> [p, N, C] partition-major host shuffle."""
    n = a.shape[0] // p
    return np.ascontiguousarray(a.reshape(n, p, -1).transpose(1, 0, 2))


def make_in_maps(inputs):
    x = np.asarray(inputs["x"], np.float32).reshape(T, D)
    Wq = np.asarray(inputs["Wq"], np.float32)
    Wk = np.asarray(inputs["Wk"], np.float32)
    Wv = np.asarray(inputs["Wv"], np.float32)
    Wproj = np.asarray(inputs["Wproj"], np.float32)
    bproj = np.asarray(inputs["bproj"], np.float32).reshape(1, D)
    W1 = np.asarray(inputs["W1"], np.float32)
    W2 = np.asarray(inputs["W2"], np.float32)
    g1 = np.asarray(inputs["g1"], np.float32)
    g2 = np.asarray(inputs["g2"], np.float32)

    xT = _shuf(np.ascontiguousarray(x.T).astype(np.float16))          # [128,8,T]
    Wq_f = (Wq * g1[None, :, None]).astype(np.float16)
    Wk_f = (Wk * g1[None, :, None]).astype(np.float16)
    Wv_f = (Wv * g1[None, :, None]).astype(np.float16)
    Wp_s = _shuf(Wproj.astype(np.float16))                            # [128,8,D]
    W1_f = (W1 * g2[:, None]).astype(np.float16)
    # w1: [p, g, d, 512] so each g-block is one contiguous DMA
    w1_s = np.ascontiguousarray(
        W1_f.reshape(ND, 128, NG, 512).transpose(1, 2, 0, 3)
    )
    # w2: [p, g, j, D]
    w2_s = np.ascontiguousarray(
        W2.astype(np.float16).reshape(NG, 4, 128, D).transpose(2, 0, 1, 3)
    )

    common = {
        "xT": xT,
        "wp": Wp_s,
        "bp": bproj.astype(np.float16),
        "w1": w1_s,
        "w2": w2_s,
        "ident": np.eye(128, dtype=np.float16),
        "ones_c": np.ones((128, 1), np.float16),
        "ones_r": np.ones((1, 128), np.float16),
        "onescol": np.ones((128, NT), np.float16),
        "epsb": np.full((128, 1), EPS, np.float32),
    }
    in_maps = []
    for c in range(NCORES):
        heads = [HPC * c + h for h in range(HPC)]
        in_maps.append(
            {
                **common,
                "xch": np.ascontiguousarray(x[_chunk_rows(c)]),
                "wq": _shuf(np.concatenate([Wq_f[h] for h in heads], 1)),
                "wk": _shuf(np.concatenate([Wk_f[h] for h in heads], 1)),
                "wv": _shuf(np.concatenate([Wv_f[h] for h in heads], 1)),
            }
        )
    return in_maps


def _chunk_rows(j):
    """Core j owns rows [1024m + 128j : +128) for m = 0, 1."""
    return np.concatenate(
        [np.arange(1024 * m + 128 * j, 1024 * m + 128 * (j + 1)) for m in range(2)]
    )


def run(inputs, **kwargs):
    nc = build_nc()
    in_maps = make_in_maps(inputs)
    res = bass_utils.run_bass_kernel_spmd(
        nc, in_maps, core_ids=list(range(NCORES)), **kwargs
    )
    out = np.empty((T, D), np.float32)
    for c in range(NCORES):
        out[_chunk_rows(c)] = res.results[c]["out"]
    return out.reshape(1, T, D), res


def kernel(**inputs):
    out, _ = run(inputs)
    return out
